# revision 27
# baseline (speedup 1.0000x reference)
"""MoE layer (N=8192, D=512, H=2048, E=8, top-2) on 8 TRN2 NeuronCores.

Strategy: data-parallel over tokens (1024 tokens/core) with host-side top-2
routing. The host computes the (tiny) gating softmax/top-2, gathers each
core's tokens per expert into capacity-padded buckets (C slots, weight-0
padding), and builds a sparse combine matrix M[slot, token] holding the
renormalized top-2 weights. The device then does only the routed expert
compute (~2.7x less matmul work than dense all-expert):

  - mm1: psum[Hcols=128, slot=C] += W1[Dk,Hcols].T @ xgT[Dk, slot]
    (hidden kept transposed: both matmuls consume W1/W2 in natural layout,
    no on-device transposes)
  - LN-over-H: mean analytically via x @ rowmean(W1) (host-precomputed),
    E[h^2] via ones-vector matmuls (partition reduction on PE)
  - mm2: psum[slot=128, D=512] += a[Hk, slot].T @ W2[Hk, D]
  - combine: out_psum[tok, D] = sum_kt M[kt, tok].T @ y[kt, D]  (+ b2 rows
    with gating weights as an extra K=8 term) -- the scatter-add is a matmul
  - final LayerNorm reads the combine PSUM directly

Matmuls in bf16 (fp32 accumulate), everything else fp32. The aux loss
(scalar stats over the full gating probabilities) is computed on host.
"""

import numpy as np
import ml_dtypes

N, D, H, E, K = 8192, 512, 2048, 8, 2
NCORES = 8
NS = N // NCORES  # tokens per core
EPS_LN = 1e-5
EPS_AUX = 1e-6

_CACHE = {}


def _build(C):
    """C = per-(core,expert) capacity, multiple of 128."""
    import concourse.bass as bass
    from concourse import bacc
    import concourse.mybir as mybir
    from concourse.tile import TileContext

    F32 = mybir.dt.float32
    BF16 = mybir.dt.bfloat16
    AF = mybir.ActivationFunctionType
    ALU = mybir.AluOpType

    KD = D // 128        # 4 contraction tiles for mm1
    HT = H // 128        # 16 hidden tiles
    NTOK = NS // 128     # 8 token tiles of 128
    assert C % 64 == 0 and (E * C) % 128 == 0
    NSLOT = (E * C) // 128   # total y slot-tiles (combine contraction tiles)

    def bcast(ap, p=128):
        return bass.AP(tensor=ap.tensor, offset=ap.offset, ap=[[0, p]] + list(ap.ap))

    nc = bacc.Bacc(None, target_bir_lowering=False)

    xgT = nc.dram_tensor("xgT", [E, D, C], BF16, kind="ExternalInput")
    w1 = nc.dram_tensor("w1", [E, D, H], BF16, kind="ExternalInput")
    b1 = nc.dram_tensor("b1", [E, H], F32, kind="ExternalInput")
    lng = nc.dram_tensor("lng", [E, H], F32, kind="ExternalInput")
    lnb = nc.dram_tensor("lnb", [E, H], F32, kind="ExternalInput")
    w2 = nc.dram_tensor("w2", [E, H, D], BF16, kind="ExternalInput")
    b2 = nc.dram_tensor("b2", [E, D], BF16, kind="ExternalInput")
    w1m = nc.dram_tensor("w1m", [E, D], BF16, kind="ExternalInput")
    b1m = nc.dram_tensor("b1m", [E], F32, kind="ExternalInput")
    cm = nc.dram_tensor("cm", [E * C, NS], BF16, kind="ExternalInput")
    cmb2 = nc.dram_tensor("cmb2", [E, NS], BF16, kind="ExternalInput")
    og = nc.dram_tensor("og", [D], F32, kind="ExternalInput")
    ob = nc.dram_tensor("ob", [D], F32, kind="ExternalInput")

    out = nc.dram_tensor("out", [NS, D], F32, kind="ExternalOutput")

    with TileContext(nc) as tc:
        with (
            tc.tile_pool(name="consts", bufs=1) as consts,
            tc.tile_pool(name="wpool", bufs=2) as wpool,
            tc.tile_pool(name="w2pool", bufs=3) as w2pool,
            tc.tile_pool(name="hpool", bufs=3) as hpool,
            tc.tile_pool(name="ypool", bufs=1) as ypool,
            tc.tile_pool(name="mpool", bufs=2) as mpool,
            tc.tile_pool(name="work", bufs=3) as work,
            tc.tile_pool(name="hsqp", bufs=4) as hsqp,
            tc.tile_pool(name="rows", bufs=2) as rows,
            tc.tile_pool(name="bcastp", bufs=2) as bcastp,
            tc.tile_pool(name="psA", bufs=2, space="PSUM") as psA,
            tc.tile_pool(name="psStats", bufs=1, space="PSUM") as psStats,
            tc.tile_pool(name="psY", bufs=2, space="PSUM") as psY,
            tc.tile_pool(name="psC", bufs=2, space="PSUM") as psC,
        ):
            # ---------- constants ----------
            og_b = consts.tile([128, D], F32, tag="og")
            nc.sync.dma_start(out=og_b, in_=bcast(og.ap()))
            ob_b = consts.tile([128, D], F32, tag="ob")
            nc.sync.dma_start(out=ob_b, in_=bcast(ob.ap()))
            ones = consts.tile([128, 1], BF16, tag="ones")
            nc.vector.memset(ones, 1.0 / H)  # 2^-11, exact in bf16
            eps_t = consts.tile([128, 1], F32, tag="eps")
            nc.vector.memset(eps_t, EPS_LN)
            b1msb = consts.tile([1, E], F32, tag="b1m")
            nc.sync.dma_start(out=b1msb, in_=bcast(b1m.ap(), p=1))
            b2sb = consts.tile([E, D], BF16, tag="b2")
            nc.sync.dma_start(out=b2sb, in_=b2[:, :])
            cmb2sb = consts.tile([E, NS], BF16, tag="cmb2")
            nc.sync.dma_start(out=cmb2sb, in_=cmb2[:, :])

            y_tiles = []
            for i in range(NSLOT):
                y_tiles.append(
                    consts.tile([128, D], BF16, tag=f"y_{i}", name=f"y_{i}")
                )

            # ---------- experts (software-pipelined: mm2 of expert e-1
            # is emitted during expert e so the PE queue never stalls) ----

            def emit_mm2(unit):
                if unit is None:
                    return
                ee, hts, w2t = unit
                g0 = ee * C
                pos = g0
                while pos < g0 + C:
                    nxt = min(g0 + C, (pos // 128 + 1) * 128)
                    sz = nxt - pos
                    ls = pos - g0
                    y_ps = psY.tile([128, D], F32, tag="y")
                    for hk in range(HT):
                        nc.tensor.matmul(
                            y_ps[:sz, :],
                            hts[hk][:, ls:ls + sz],
                            w2t[hk // (HT // 2)][:, hk % (HT // 2), :],
                            start=(hk == 0),
                            stop=(hk == HT - 1),
                        )
                    po = pos % 128
                    nc.vector.tensor_copy(
                        y_tiles[pos // 128][po:po + sz, :], y_ps[:sz, :]
                    )
                    pos = nxt

            def emit_stage2(st):
                # stats + normalize for expert st (runs one block later)
                ee = st["e"]
                nc.tensor.matmul(st["sq_ps"], ones, st["sq_acc"], start=True, stop=True)
                mrow = rows.tile([1, C], F32, tag="mrow")
                nc.vector.tensor_scalar(
                    mrow, st["mean_ps"], b1msb[:, ee:ee + 1], None, ALU.add
                )
                rtmp = rows.tile([1, C], F32, tag="rtmp")
                nc.vector.tensor_mul(rtmp, mrow, mrow)
                nc.vector.tensor_sub(rtmp, st["sq_ps"], rtmp)  # var
                rstd0 = rows.tile([1, C], F32, tag="rstd0")
                nc.scalar.activation(rstd0, rtmp, AF.Sqrt, bias=eps_t[:1, :])
                rrstd = rows.tile([1, C], F32, tag="rrstd")
                nc.vector.reciprocal(rrstd, rstd0)
                r_row = rows.tile([1, C], BF16, tag="rrow")
                nc.vector.tensor_copy(r_row, rrstd)
                m2_row = rows.tile([1, C], BF16, tag="m2row")
                nc.vector.tensor_mul(m2_row, mrow, rrstd)
                r_b = bcastp.tile([128, C], BF16, tag="rb")
                nc.gpsimd.partition_broadcast(r_b, r_row)
                m2_b = bcastp.tile([128, C], BF16, tag="m2b")
                nc.gpsimd.partition_broadcast(m2_b, m2_row)
                for hi in range(HT):
                    h_sb = st["h_tiles"][hi]
                    t = work.tile([128, C], BF16, tag="t1")
                    nc.vector.tensor_mul(t, h_sb, r_b)
                    nc.vector.tensor_sub(h_sb, t, m2_b)
                    nc.scalar.activation(
                        h_sb,
                        h_sb,
                        AF.Gelu,
                        bias=st["lnbsb"][:, hi:hi + 1],
                        scale=st["lngsb"][:, hi:hi + 1],
                    )
                return (ee, st["h_tiles"], st["w2sb"])

            pending = None    # stage-1 done (mm1/hsq), awaiting stats+norm
            prev_unit = None  # stage-2 done, awaiting mm2
            for e in range(E):
                w1sb = []
                for k in range(KD):
                    t = wpool.tile([128, H], BF16, tag=f"w1_{k}", name=f"w1_{k}")
                    nc.sync.dma_start(
                        out=t, in_=w1[e, k * 128:(k + 1) * 128, :]
                    )
                    w1sb.append(t)
                xgsb = []
                for k in range(KD):
                    t = wpool.tile([128, C], BF16, tag=f"xg_{k}", name=f"xg_{k}")
                    nc.sync.dma_start(
                        out=t, in_=xgT[e, k * 128:(k + 1) * 128, :]
                    )
                    xgsb.append(t)
                b1sb = wpool.tile([128, HT], F32, tag="b1")
                nc.sync.dma_start(out=b1sb, in_=b1[e].rearrange("(i p) -> p i", p=128))
                lngsb = wpool.tile([128, HT], F32, tag="lng")
                nc.sync.dma_start(out=lngsb, in_=lng[e].rearrange("(i p) -> p i", p=128))
                lnbsb = wpool.tile([128, HT], F32, tag="lnb")
                nc.sync.dma_start(out=lnbsb, in_=lnb[e].rearrange("(i p) -> p i", p=128))
                w1msb = wpool.tile([128, KD], BF16, tag="w1m")
                nc.sync.dma_start(out=w1msb, in_=w1m[e].rearrange("(k p) -> p k", p=128))
                w2sb = []
                for half in range(2):
                    t = w2pool.tile(
                        [128, HT // 2, D], BF16, tag=f"w2_{half}", name=f"w2_{half}"
                    )
                    nc.sync.dma_start(
                        out=t,
                        in_=w2[e, half * (H // 2):(half + 1) * (H // 2), :].rearrange(
                            "(t p) d -> p t d", p=128
                        ),
                    )
                    w2sb.append(t)

                mean_ps = psStats.tile([1, C], F32, tag="sum")
                sq_ps = psStats.tile([1, C], F32, tag="sq")
                h_tiles = []
                sq_acc = hsqp.tile([128, C], BF16, tag="sqacc", bufs=2)

                for hi in range(HT):
                    h_ps = psA.tile([128, C], F32, tag="ps")
                    for k in range(KD):
                        nc.tensor.matmul(
                            h_ps,
                            w1sb[k][:, hi * 128:(hi + 1) * 128],
                            xgsb[k],
                            start=(k == 0),
                            stop=(k == KD - 1),
                        )
                    h_sb = hpool.tile([128, C], BF16, tag=f"h{hi}")
                    nc.scalar.activation(
                        h_sb, h_ps, AF.Identity, bias=b1sb[:, hi:hi + 1]
                    )
                    if hi == 0:
                        nc.vector.tensor_mul(sq_acc, h_sb, h_sb)
                    else:
                        hsq = hsqp.tile([128, C], BF16, tag="hsq")
                        nc.vector.tensor_mul(hsq, h_sb, h_sb)
                        nc.vector.tensor_add(sq_acc, sq_acc, hsq)
                    h_tiles.append(h_sb)

                unit = emit_stage2(pending) if pending is not None else None

                # mean = xg @ mean_H(W1[e]) + mean(b1[e])  (no h dependency)
                for k in range(KD):
                    nc.tensor.matmul(
                        mean_ps,
                        w1msb[:, k:k + 1],
                        xgsb[k],
                        start=(k == 0),
                        stop=(k == KD - 1),
                    )

                emit_mm2(prev_unit)
                prev_unit = unit
                pending = {
                    "e": e, "h_tiles": h_tiles, "sq_acc": sq_acc,
                    "mean_ps": mean_ps, "sq_ps": sq_ps,
                    "lngsb": lngsb, "lnbsb": lnbsb, "w2sb": w2sb,
                }

            last_unit = emit_stage2(pending)
            emit_mm2(prev_unit)
            emit_mm2(last_unit)

            emit_mm2(prev_unit)

            # ---------- combine (scatter-add as matmul) + final LN ----------
            for g in range(NTOK):
                msb = mpool.tile([128, NSLOT, 128], BF16, tag="m")
                nc.sync.dma_start(
                    out=msb,
                    in_=cm.ap().rearrange("(kt p) t -> p kt t", p=128)[
                        :, :, g * 128:(g + 1) * 128
                    ],
                )
                o_ps = psC.tile([128, D], F32, tag="oc")
                for kt in range(NSLOT):
                    nc.tensor.matmul(
                        o_ps,
                        msb[:, kt, :],
                        y_tiles[kt],
                        start=(kt == 0),
                        stop=False,
                    )
                # + b2 rows weighted by gating weights (K=E matmul)
                nc.tensor.matmul(
                    o_ps,
                    cmb2sb[:, g * 128:(g + 1) * 128],
                    b2sb,
                    start=False,
                    stop=True,
                )
                st6 = work.tile([128, 6], F32, tag="fst6")
                nc.vector.bn_stats(st6, o_ps)
                mv = work.tile([128, 2], F32, tag="fmv")
                nc.vector.bn_aggr(mv, st6)
                stdf = work.tile([128, 1], F32, tag="fstd")
                nc.scalar.activation(stdf, mv[:, 1:2], AF.Sqrt, bias=eps_t)
                rf = work.tile([128, 1], F32, tag="frf")
                nc.vector.reciprocal(rf, stdf)
                t = work.tile([128, D], F32, tag="fin")
                nc.vector.tensor_scalar(
                    t, o_ps, mv[:, 0:1], rf, ALU.subtract, ALU.mult
                )
                nc.vector.tensor_mul(t, t, og_b)
                nc.vector.tensor_add(t, t, ob_b)
                nc.sync.dma_start(out=out[g * 128:(g + 1) * 128, :], in_=t)

    nc.compile()
    return nc


def _get_nc(C):
    key = ("nc", C)
    if key not in _CACHE:
        _CACHE[key] = _build(C)
    return _CACHE[key]


def kernel(x, gate_W, gate_b, W1, b1, ln_g, ln_b, W2, b2, out_g, out_b):
    import os
    from concourse.bass_utils import run_bass_kernel_spmd

    x = np.asarray(x, dtype=np.float32)
    gate_W = np.asarray(gate_W, dtype=np.float32)
    gate_b = np.asarray(gate_b, dtype=np.float32)

    # ---------- host gating: softmax + top-2 + renormalize ----------
    logits = x @ gate_W + gate_b                      # [N, E] fp32
    lmax = logits.max(axis=1, keepdims=True)
    ex = np.exp((logits - lmax).astype(np.float32))
    probs = ex / ex.sum(axis=1, keepdims=True)        # [N, E] fp32
    order = np.argsort(-probs, axis=1, kind="stable")
    top_idx = order[:, :K]                            # [N, 2]
    top_p = np.take_along_axis(probs, top_idx, axis=1)
    top_w = (top_p / top_p.sum(axis=1, keepdims=True)).astype(np.float32)

    # aux loss (host; matches reference formulas)
    imp = probs.sum(axis=0).astype(np.float64)
    mask_count = np.zeros(E, dtype=np.float64)
    for kk in range(K):
        mask_count += np.bincount(top_idx[:, kk], minlength=E)
    load = mask_count / N

    def _loss(v):
        return (np.std(v, ddof=1) / (np.mean(v) + EPS_AUX)) ** 2

    aux = np.float32(_loss(imp) + _loss(load))

    # ---------- capacity ----------
    counts = np.zeros((NCORES, E), dtype=np.int64)
    for c in range(NCORES):
        ti = top_idx[c * NS:(c + 1) * NS]
        for e in range(E):
            counts[c, e] = int((ti == e).sum())
    C = max(320, int(np.ceil(counts.max() / 64.0) * 64))
    while (E * C) % 128 != 0:
        C += 64

    nc = _get_nc(C)

    # ---------- per-core routing buffers ----------
    bf16 = ml_dtypes.bfloat16
    W1f = np.asarray(W1, dtype=np.float32)
    b1f = np.ascontiguousarray(np.asarray(b1, dtype=np.float32))
    common = {
        "w1": np.ascontiguousarray(W1f).astype(bf16),
        "b1": b1f,
        "lng": np.ascontiguousarray(np.asarray(ln_g, dtype=np.float32)),
        "lnb": np.ascontiguousarray(np.asarray(ln_b, dtype=np.float32)),
        "w2": np.ascontiguousarray(np.asarray(W2, dtype=np.float32)).astype(bf16),
        "b2": np.ascontiguousarray(np.asarray(b2, dtype=np.float32)).astype(bf16),
        "w1m": np.ascontiguousarray(W1f.mean(axis=2)).astype(bf16),
        "b1m": np.ascontiguousarray(b1f.mean(axis=1)),
        "og": np.ascontiguousarray(np.asarray(out_g, dtype=np.float32)),
        "ob": np.ascontiguousarray(np.asarray(out_b, dtype=np.float32)),
    }

    in_maps = []
    for c in range(NCORES):
        sl = slice(c * NS, (c + 1) * NS)
        xs = x[sl]                                    # [NS, D]
        ti = top_idx[sl]                              # [NS, 2]
        tw = top_w[sl]
        xgT_c = np.zeros((E, D, C), dtype=bf16)
        cm_c = np.zeros((E * C, NS), dtype=bf16)
        cmb2_c = np.zeros((E, NS), dtype=bf16)
        for e in range(E):
            rows_e, which = np.nonzero(ti == e)
            ne = rows_e.shape[0]
            assert ne <= C, f"capacity overflow: {ne} > {C}"
            xgT_c[e, :, :ne] = xs[rows_e].T.astype(bf16)
            w = tw[rows_e, which].astype(bf16)
            cm_c[e * C + np.arange(ne), rows_e] = w
            cmb2_c[e, rows_e] = w
        in_maps.append(
            {**common, "xgT": xgT_c, "cm": cm_c, "cmb2": cmb2_c}
        )

    trace = bool(int(os.environ.get("BASS_KERNEL_TRACE", "0")))
    if trace:
        _install_ntff_hook()
    res = run_bass_kernel_spmd(
        nc, in_maps, core_ids=list(range(NCORES)), trace=trace
    )
    _CACHE["exec_time_ns"] = res.exec_time_ns

    out = np.concatenate([res.results[c]["out"] for c in range(NCORES)], axis=0)
    return out, aux


def _install_ntff_hook():
    import sys
    import types

    if "antenv.axon_hooks" in sys.modules:
        return
    mod = types.ModuleType("antenv.axon_hooks")
    hook = [None]
    mod.set_axon_ntff_profile_hook = lambda h: hook.__setitem__(0, h)
    mod.get_axon_ntff_profile_hook = lambda: hook[0]
    sys.modules["antenv.axon_hooks"] = mod
    try:
        import antenv

        antenv.axon_hooks = mod
        from trn_agent_boot.trn_boot import _ntff_profile_via_ctypes

        mod.set_axon_ntff_profile_hook(
            _ntff_profile_via_ctypes("/opt/axon/libaxon_pjrt.so")
        )
    except Exception:
        pass


# revision 28
# speedup vs baseline: 1.0097x; 1.0097x over previous
"""MoE layer (N=8192, D=512, H=2048, E=8, top-2) on 8 TRN2 NeuronCores.

Strategy: data-parallel over tokens (1024 tokens/core) with host-side top-2
routing. The host computes the (tiny) gating softmax/top-2, gathers each
core's tokens per expert into capacity-padded buckets (C slots, weight-0
padding), and builds a sparse combine matrix M[slot, token] holding the
renormalized top-2 weights. The device then does only the routed expert
compute (~2.7x less matmul work than dense all-expert):

  - mm1: psum[Hcols=128, slot=C] += W1[Dk,Hcols].T @ xgT[Dk, slot]
    (hidden kept transposed: both matmuls consume W1/W2 in natural layout,
    no on-device transposes)
  - LN-over-H: mean analytically via x @ rowmean(W1) (host-precomputed),
    E[h^2] via ones-vector matmuls (partition reduction on PE)
  - mm2: psum[slot=128, D=512] += a[Hk, slot].T @ W2[Hk, D]
  - combine: out_psum[tok, D] = sum_kt M[kt, tok].T @ y[kt, D]  (+ b2 rows
    with gating weights as an extra K=8 term) -- the scatter-add is a matmul
  - final LayerNorm reads the combine PSUM directly

Matmuls in bf16 (fp32 accumulate), everything else fp32. The aux loss
(scalar stats over the full gating probabilities) is computed on host.
"""

import numpy as np
import ml_dtypes

N, D, H, E, K = 8192, 512, 2048, 8, 2
NCORES = 8
NS = N // NCORES  # tokens per core
EPS_LN = 1e-5
EPS_AUX = 1e-6

_CACHE = {}


def _build(C):
    """C = per-(core,expert) capacity, multiple of 128."""
    import concourse.bass as bass
    from concourse import bacc
    import concourse.mybir as mybir
    from concourse.tile import TileContext

    F32 = mybir.dt.float32
    BF16 = mybir.dt.bfloat16
    AF = mybir.ActivationFunctionType
    ALU = mybir.AluOpType

    KD = D // 128        # 4 contraction tiles for mm1
    HT = H // 128        # 16 hidden tiles
    NTOK = NS // 128     # 8 token tiles of 128
    assert C % 64 == 0 and (E * C) % 128 == 0
    NSLOT = (E * C) // 128   # total y slot-tiles (combine contraction tiles)

    def bcast(ap, p=128):
        return bass.AP(tensor=ap.tensor, offset=ap.offset, ap=[[0, p]] + list(ap.ap))

    nc = bacc.Bacc(None, target_bir_lowering=False)

    xgT = nc.dram_tensor("xgT", [E, D, C], BF16, kind="ExternalInput")
    w1 = nc.dram_tensor("w1", [E, D, H], BF16, kind="ExternalInput")
    b1 = nc.dram_tensor("b1", [E, H], F32, kind="ExternalInput")
    lng = nc.dram_tensor("lng", [E, H], F32, kind="ExternalInput")
    lnb = nc.dram_tensor("lnb", [E, H], F32, kind="ExternalInput")
    w2 = nc.dram_tensor("w2", [E, H, D], BF16, kind="ExternalInput")
    b2 = nc.dram_tensor("b2", [E, D], BF16, kind="ExternalInput")
    w1m = nc.dram_tensor("w1m", [E, D], BF16, kind="ExternalInput")
    b1m = nc.dram_tensor("b1m", [E], F32, kind="ExternalInput")
    cm = nc.dram_tensor("cm", [E * C, NS], BF16, kind="ExternalInput")
    cmb2 = nc.dram_tensor("cmb2", [E, NS], BF16, kind="ExternalInput")
    og = nc.dram_tensor("og", [D], F32, kind="ExternalInput")
    ob = nc.dram_tensor("ob", [D], F32, kind="ExternalInput")

    out = nc.dram_tensor("out", [NS, D], F32, kind="ExternalOutput")

    with TileContext(nc) as tc:
        with (
            tc.tile_pool(name="consts", bufs=1) as consts,
            tc.tile_pool(name="wpool", bufs=2) as wpool,
            tc.tile_pool(name="w2pool", bufs=3) as w2pool,
            tc.tile_pool(name="hpool", bufs=3) as hpool,
            tc.tile_pool(name="ypool", bufs=1) as ypool,
            tc.tile_pool(name="mpool", bufs=2) as mpool,
            tc.tile_pool(name="work", bufs=3) as work,
            tc.tile_pool(name="hsqp", bufs=4) as hsqp,
            tc.tile_pool(name="rows", bufs=2) as rows,
            tc.tile_pool(name="bcastp", bufs=2) as bcastp,
            tc.tile_pool(name="psA", bufs=2, space="PSUM") as psA,
            tc.tile_pool(name="psStats", bufs=1, space="PSUM") as psStats,
            tc.tile_pool(name="psY", bufs=2, space="PSUM") as psY,
            tc.tile_pool(name="psC", bufs=2, space="PSUM") as psC,
        ):
            # ---------- constants ----------
            og_b = consts.tile([128, D], F32, tag="og")
            nc.sync.dma_start(out=og_b, in_=bcast(og.ap()))
            ob_b = consts.tile([128, D], F32, tag="ob")
            nc.sync.dma_start(out=ob_b, in_=bcast(ob.ap()))
            ones = consts.tile([128, 1], BF16, tag="ones")
            nc.vector.memset(ones, 1.0 / H)  # 2^-11, exact in bf16
            eps_t = consts.tile([128, 1], F32, tag="eps")
            nc.vector.memset(eps_t, EPS_LN)
            b1msb = consts.tile([1, E], F32, tag="b1m")
            nc.sync.dma_start(out=b1msb, in_=bcast(b1m.ap(), p=1))
            b2sb = consts.tile([E, D], BF16, tag="b2")
            nc.sync.dma_start(out=b2sb, in_=b2[:, :])
            cmb2sb = consts.tile([E, NS], BF16, tag="cmb2")
            nc.sync.dma_start(out=cmb2sb, in_=cmb2[:, :])

            y_tiles = []
            for i in range(NSLOT):
                y_tiles.append(
                    consts.tile([128, D], BF16, tag=f"y_{i}", name=f"y_{i}")
                )

            # ---------- experts (software-pipelined: mm2 of expert e-1
            # is emitted during expert e so the PE queue never stalls) ----

            def emit_mm2(unit):
                if unit is None:
                    return
                ee, hts, w2t = unit
                g0 = ee * C
                pos = g0
                while pos < g0 + C:
                    nxt = min(g0 + C, (pos // 128 + 1) * 128)
                    sz = nxt - pos
                    ls = pos - g0
                    y_ps = psY.tile([128, D], F32, tag="y")
                    for hk in range(HT):
                        nc.tensor.matmul(
                            y_ps[:sz, :],
                            hts[hk][:, ls:ls + sz],
                            w2t[:, hk, :],
                            start=(hk == 0),
                            stop=(hk == HT - 1),
                        )
                    po = pos % 128
                    nc.vector.tensor_copy(
                        y_tiles[pos // 128][po:po + sz, :], y_ps[:sz, :]
                    )
                    pos = nxt

            def emit_stage2(st):
                # stats + normalize for expert st (runs one block later)
                ee = st["e"]
                nc.tensor.matmul(st["sq_ps"], ones, st["sq_acc"], start=True, stop=True)
                mrow = rows.tile([1, C], F32, tag="mrow")
                nc.vector.tensor_scalar(
                    mrow, st["mean_ps"], b1msb[:, ee:ee + 1], None, ALU.add
                )
                rtmp = rows.tile([1, C], F32, tag="rtmp")
                nc.vector.tensor_mul(rtmp, mrow, mrow)
                nc.vector.tensor_sub(rtmp, st["sq_ps"], rtmp)  # var
                rstd0 = rows.tile([1, C], F32, tag="rstd0")
                nc.scalar.activation(rstd0, rtmp, AF.Sqrt, bias=eps_t[:1, :])
                rrstd = rows.tile([1, C], F32, tag="rrstd")
                nc.vector.reciprocal(rrstd, rstd0)
                r_row = rows.tile([1, C], BF16, tag="rrow")
                nc.vector.tensor_copy(r_row, rrstd)
                m2_row = rows.tile([1, C], BF16, tag="m2row")
                nc.vector.tensor_mul(m2_row, mrow, rrstd)
                r_b = bcastp.tile([128, C], BF16, tag="rb")
                nc.gpsimd.partition_broadcast(r_b, r_row)
                m2_b = bcastp.tile([128, C], BF16, tag="m2b")
                nc.gpsimd.partition_broadcast(m2_b, m2_row)
                for hi in range(HT):
                    h_sb = st["h_tiles"][hi]
                    t = work.tile([128, C], BF16, tag="t1")
                    nc.vector.tensor_mul(t, h_sb, r_b)
                    nc.vector.tensor_sub(h_sb, t, m2_b)
                    nc.scalar.activation(
                        h_sb,
                        h_sb,
                        AF.Gelu,
                        bias=st["lnbsb"][:, hi:hi + 1],
                        scale=st["lngsb"][:, hi:hi + 1],
                    )
                return (ee, st["h_tiles"], st["w2sb"])

            pending = None    # stage-1 done (mm1/hsq), awaiting stats+norm
            prev_unit = None  # stage-2 done, awaiting mm2
            for e in range(E):
                w1sb = []
                for k in range(KD):
                    t = wpool.tile([128, H], BF16, tag=f"w1_{k}", name=f"w1_{k}")
                    nc.sync.dma_start(
                        out=t, in_=w1[e, k * 128:(k + 1) * 128, :]
                    )
                    w1sb.append(t)
                xgsb = []
                for k in range(KD):
                    t = wpool.tile([128, C], BF16, tag=f"xg_{k}", name=f"xg_{k}")
                    nc.sync.dma_start(
                        out=t, in_=xgT[e, k * 128:(k + 1) * 128, :]
                    )
                    xgsb.append(t)
                b1sb = wpool.tile([128, HT], F32, tag="b1")
                nc.sync.dma_start(out=b1sb, in_=b1[e].rearrange("(i p) -> p i", p=128))
                lngsb = wpool.tile([128, HT], F32, tag="lng")
                nc.sync.dma_start(out=lngsb, in_=lng[e].rearrange("(i p) -> p i", p=128))
                lnbsb = wpool.tile([128, HT], F32, tag="lnb")
                nc.sync.dma_start(out=lnbsb, in_=lnb[e].rearrange("(i p) -> p i", p=128))
                w1msb = wpool.tile([128, KD], BF16, tag="w1m")
                nc.sync.dma_start(out=w1msb, in_=w1m[e].rearrange("(k p) -> p k", p=128))
                w2sb = w2pool.tile([128, HT, D], BF16, tag="w2")
                nc.sync.dma_start(
                    out=w2sb, in_=w2[e].rearrange("(t p) d -> p t d", p=128)
                )

                mean_ps = psStats.tile([1, C], F32, tag="sum")
                sq_ps = psStats.tile([1, C], F32, tag="sq")
                h_tiles = []
                sq_acc = hsqp.tile([128, C], BF16, tag="sqacc", bufs=2)

                for hi in range(HT):
                    h_ps = psA.tile([128, C], F32, tag="ps")
                    for k in range(KD):
                        nc.tensor.matmul(
                            h_ps,
                            w1sb[k][:, hi * 128:(hi + 1) * 128],
                            xgsb[k],
                            start=(k == 0),
                            stop=(k == KD - 1),
                        )
                    h_sb = hpool.tile([128, C], BF16, tag=f"h{hi}")
                    nc.scalar.activation(
                        h_sb, h_ps, AF.Identity, bias=b1sb[:, hi:hi + 1]
                    )
                    if hi == 0:
                        nc.vector.tensor_mul(sq_acc, h_sb, h_sb)
                    else:
                        hsq = hsqp.tile([128, C], BF16, tag="hsq")
                        nc.vector.tensor_mul(hsq, h_sb, h_sb)
                        nc.vector.tensor_add(sq_acc, sq_acc, hsq)
                    h_tiles.append(h_sb)

                unit = emit_stage2(pending) if pending is not None else None

                # mean = xg @ mean_H(W1[e]) + mean(b1[e])  (no h dependency)
                for k in range(KD):
                    nc.tensor.matmul(
                        mean_ps,
                        w1msb[:, k:k + 1],
                        xgsb[k],
                        start=(k == 0),
                        stop=(k == KD - 1),
                    )

                emit_mm2(prev_unit)
                prev_unit = unit
                pending = {
                    "e": e, "h_tiles": h_tiles, "sq_acc": sq_acc,
                    "mean_ps": mean_ps, "sq_ps": sq_ps,
                    "lngsb": lngsb, "lnbsb": lnbsb, "w2sb": w2sb,
                }

            last_unit = emit_stage2(pending)
            emit_mm2(prev_unit)
            emit_mm2(last_unit)

            emit_mm2(prev_unit)

            # ---------- combine (scatter-add as matmul) + final LN ----------
            for g in range(NTOK):
                msb = mpool.tile([128, NSLOT, 128], BF16, tag="m")
                nc.sync.dma_start(
                    out=msb,
                    in_=cm.ap().rearrange("(kt p) t -> p kt t", p=128)[
                        :, :, g * 128:(g + 1) * 128
                    ],
                )
                o_ps = psC.tile([128, D], F32, tag="oc")
                for kt in range(NSLOT):
                    nc.tensor.matmul(
                        o_ps,
                        msb[:, kt, :],
                        y_tiles[kt],
                        start=(kt == 0),
                        stop=False,
                    )
                # + b2 rows weighted by gating weights (K=E matmul)
                nc.tensor.matmul(
                    o_ps,
                    cmb2sb[:, g * 128:(g + 1) * 128],
                    b2sb,
                    start=False,
                    stop=True,
                )
                st6 = work.tile([128, 6], F32, tag="fst6")
                nc.vector.bn_stats(st6, o_ps)
                mv = work.tile([128, 2], F32, tag="fmv")
                nc.vector.bn_aggr(mv, st6)
                stdf = work.tile([128, 1], F32, tag="fstd")
                nc.scalar.activation(stdf, mv[:, 1:2], AF.Sqrt, bias=eps_t)
                rf = work.tile([128, 1], F32, tag="frf")
                nc.vector.reciprocal(rf, stdf)
                t = work.tile([128, D], F32, tag="fin")
                nc.vector.tensor_scalar(
                    t, o_ps, mv[:, 0:1], rf, ALU.subtract, ALU.mult
                )
                nc.vector.tensor_mul(t, t, og_b)
                nc.vector.tensor_add(t, t, ob_b)
                nc.sync.dma_start(out=out[g * 128:(g + 1) * 128, :], in_=t)

    nc.compile()
    return nc


def _get_nc(C):
    key = ("nc", C)
    if key not in _CACHE:
        _CACHE[key] = _build(C)
    return _CACHE[key]


def kernel(x, gate_W, gate_b, W1, b1, ln_g, ln_b, W2, b2, out_g, out_b):
    import os
    from concourse.bass_utils import run_bass_kernel_spmd

    x = np.asarray(x, dtype=np.float32)
    gate_W = np.asarray(gate_W, dtype=np.float32)
    gate_b = np.asarray(gate_b, dtype=np.float32)

    # ---------- host gating: softmax + top-2 + renormalize ----------
    logits = x @ gate_W + gate_b                      # [N, E] fp32
    lmax = logits.max(axis=1, keepdims=True)
    ex = np.exp((logits - lmax).astype(np.float32))
    probs = ex / ex.sum(axis=1, keepdims=True)        # [N, E] fp32
    order = np.argsort(-probs, axis=1, kind="stable")
    top_idx = order[:, :K]                            # [N, 2]
    top_p = np.take_along_axis(probs, top_idx, axis=1)
    top_w = (top_p / top_p.sum(axis=1, keepdims=True)).astype(np.float32)

    # aux loss (host; matches reference formulas)
    imp = probs.sum(axis=0).astype(np.float64)
    mask_count = np.zeros(E, dtype=np.float64)
    for kk in range(K):
        mask_count += np.bincount(top_idx[:, kk], minlength=E)
    load = mask_count / N

    def _loss(v):
        return (np.std(v, ddof=1) / (np.mean(v) + EPS_AUX)) ** 2

    aux = np.float32(_loss(imp) + _loss(load))

    # ---------- capacity ----------
    counts = np.zeros((NCORES, E), dtype=np.int64)
    for c in range(NCORES):
        ti = top_idx[c * NS:(c + 1) * NS]
        for e in range(E):
            counts[c, e] = int((ti == e).sum())
    C = max(320, int(np.ceil(counts.max() / 64.0) * 64))
    while (E * C) % 128 != 0:
        C += 64

    nc = _get_nc(C)

    # ---------- per-core routing buffers ----------
    bf16 = ml_dtypes.bfloat16
    W1f = np.asarray(W1, dtype=np.float32)
    b1f = np.ascontiguousarray(np.asarray(b1, dtype=np.float32))
    common = {
        "w1": np.ascontiguousarray(W1f).astype(bf16),
        "b1": b1f,
        "lng": np.ascontiguousarray(np.asarray(ln_g, dtype=np.float32)),
        "lnb": np.ascontiguousarray(np.asarray(ln_b, dtype=np.float32)),
        "w2": np.ascontiguousarray(np.asarray(W2, dtype=np.float32)).astype(bf16),
        "b2": np.ascontiguousarray(np.asarray(b2, dtype=np.float32)).astype(bf16),
        "w1m": np.ascontiguousarray(W1f.mean(axis=2)).astype(bf16),
        "b1m": np.ascontiguousarray(b1f.mean(axis=1)),
        "og": np.ascontiguousarray(np.asarray(out_g, dtype=np.float32)),
        "ob": np.ascontiguousarray(np.asarray(out_b, dtype=np.float32)),
    }

    in_maps = []
    for c in range(NCORES):
        sl = slice(c * NS, (c + 1) * NS)
        xs = x[sl]                                    # [NS, D]
        ti = top_idx[sl]                              # [NS, 2]
        tw = top_w[sl]
        xgT_c = np.zeros((E, D, C), dtype=bf16)
        cm_c = np.zeros((E * C, NS), dtype=bf16)
        cmb2_c = np.zeros((E, NS), dtype=bf16)
        for e in range(E):
            rows_e, which = np.nonzero(ti == e)
            ne = rows_e.shape[0]
            assert ne <= C, f"capacity overflow: {ne} > {C}"
            xgT_c[e, :, :ne] = xs[rows_e].T.astype(bf16)
            w = tw[rows_e, which].astype(bf16)
            cm_c[e * C + np.arange(ne), rows_e] = w
            cmb2_c[e, rows_e] = w
        in_maps.append(
            {**common, "xgT": xgT_c, "cm": cm_c, "cmb2": cmb2_c}
        )

    trace = bool(int(os.environ.get("BASS_KERNEL_TRACE", "0")))
    if trace:
        _install_ntff_hook()
    res = run_bass_kernel_spmd(
        nc, in_maps, core_ids=list(range(NCORES)), trace=trace
    )
    _CACHE["exec_time_ns"] = res.exec_time_ns

    out = np.concatenate([res.results[c]["out"] for c in range(NCORES)], axis=0)
    return out, aux


def _install_ntff_hook():
    import sys
    import types

    if "antenv.axon_hooks" in sys.modules:
        return
    mod = types.ModuleType("antenv.axon_hooks")
    hook = [None]
    mod.set_axon_ntff_profile_hook = lambda h: hook.__setitem__(0, h)
    mod.get_axon_ntff_profile_hook = lambda: hook[0]
    sys.modules["antenv.axon_hooks"] = mod
    try:
        import antenv

        antenv.axon_hooks = mod
        from trn_agent_boot.trn_boot import _ntff_profile_via_ctypes

        mod.set_axon_ntff_profile_hook(
            _ntff_profile_via_ctypes("/opt/axon/libaxon_pjrt.so")
        )
    except Exception:
        pass


# revision 29
# speedup vs baseline: 1.0215x; 1.0116x over previous
"""MoE layer (N=8192, D=512, H=2048, E=8, top-2) on 8 TRN2 NeuronCores.

Strategy: data-parallel over tokens (1024 tokens/core) with host-side top-2
routing. The host computes the (tiny) gating softmax/top-2, gathers each
core's tokens per expert into capacity-padded buckets (C slots, weight-0
padding), and builds a sparse combine matrix M[slot, token] holding the
renormalized top-2 weights. The device then does only the routed expert
compute (~2.7x less matmul work than dense all-expert):

  - mm1: psum[Hcols=128, slot=C] += W1[Dk,Hcols].T @ xgT[Dk, slot]
    (hidden kept transposed: both matmuls consume W1/W2 in natural layout,
    no on-device transposes)
  - LN-over-H: mean analytically via x @ rowmean(W1) (host-precomputed),
    E[h^2] via ones-vector matmuls (partition reduction on PE)
  - mm2: psum[slot=128, D=512] += a[Hk, slot].T @ W2[Hk, D]
  - combine: out_psum[tok, D] = sum_kt M[kt, tok].T @ y[kt, D]  (+ b2 rows
    with gating weights as an extra K=8 term) -- the scatter-add is a matmul
  - final LayerNorm reads the combine PSUM directly

Matmuls in bf16 (fp32 accumulate), everything else fp32. The aux loss
(scalar stats over the full gating probabilities) is computed on host.
"""

import numpy as np
import ml_dtypes

N, D, H, E, K = 8192, 512, 2048, 8, 2
NCORES = 8
NS = N // NCORES  # tokens per core
EPS_LN = 1e-5
EPS_AUX = 1e-6

_CACHE = {}


def _build(C):
    """C = per-(core,expert) capacity, multiple of 128."""
    import concourse.bass as bass
    from concourse import bacc
    import concourse.mybir as mybir
    from concourse.tile import TileContext

    F32 = mybir.dt.float32
    BF16 = mybir.dt.bfloat16
    AF = mybir.ActivationFunctionType
    ALU = mybir.AluOpType

    KD = D // 128        # 4 contraction tiles for mm1
    HT = H // 128        # 16 hidden tiles
    NTOK = NS // 128     # 8 token tiles of 128
    assert C % 64 == 0 and (E * C) % 128 == 0
    NSLOT = (E * C) // 128   # total y slot-tiles (combine contraction tiles)

    def bcast(ap, p=128):
        return bass.AP(tensor=ap.tensor, offset=ap.offset, ap=[[0, p]] + list(ap.ap))

    nc = bacc.Bacc(None, target_bir_lowering=False)

    xgT = nc.dram_tensor("xgT", [E, D, C], BF16, kind="ExternalInput")
    w1 = nc.dram_tensor("w1", [E, D, H], BF16, kind="ExternalInput")
    b1 = nc.dram_tensor("b1", [E, H], F32, kind="ExternalInput")
    lng = nc.dram_tensor("lng", [E, H], F32, kind="ExternalInput")
    lnb = nc.dram_tensor("lnb", [E, H], F32, kind="ExternalInput")
    w2 = nc.dram_tensor("w2", [E, H, D], BF16, kind="ExternalInput")
    b2 = nc.dram_tensor("b2", [E, D], BF16, kind="ExternalInput")
    w1m = nc.dram_tensor("w1m", [E, D], BF16, kind="ExternalInput")
    b1m = nc.dram_tensor("b1m", [E], F32, kind="ExternalInput")
    cm = nc.dram_tensor("cm", [E * C, NS], BF16, kind="ExternalInput")
    cmb2 = nc.dram_tensor("cmb2", [E, NS], BF16, kind="ExternalInput")
    og = nc.dram_tensor("og", [D], F32, kind="ExternalInput")
    ob = nc.dram_tensor("ob", [D], F32, kind="ExternalInput")

    out = nc.dram_tensor("out", [NS, D], F32, kind="ExternalOutput")

    with TileContext(nc) as tc:
        with (
            tc.tile_pool(name="consts", bufs=1) as consts,
            tc.tile_pool(name="wpool", bufs=2) as wpool,
            tc.tile_pool(name="w2pool", bufs=3) as w2pool,
            tc.tile_pool(name="hpool", bufs=3) as hpool,
            tc.tile_pool(name="ypool", bufs=1) as ypool,
            tc.tile_pool(name="mpool", bufs=2) as mpool,
            tc.tile_pool(name="work", bufs=3) as work,
            tc.tile_pool(name="hsqp", bufs=4) as hsqp,
            tc.tile_pool(name="rows", bufs=2) as rows,
            tc.tile_pool(name="bcastp", bufs=2) as bcastp,
            tc.tile_pool(name="psA", bufs=2, space="PSUM") as psA,
            tc.tile_pool(name="psStats", bufs=1, space="PSUM") as psStats,
            tc.tile_pool(name="psY", bufs=2, space="PSUM") as psY,
            tc.tile_pool(name="psC", bufs=2, space="PSUM") as psC,
        ):
            # ---------- constants ----------
            og_b = consts.tile([128, D], F32, tag="og")
            nc.sync.dma_start(out=og_b, in_=bcast(og.ap()))
            ob_b = consts.tile([128, D], F32, tag="ob")
            nc.sync.dma_start(out=ob_b, in_=bcast(ob.ap()))
            ones = consts.tile([128, 1], BF16, tag="ones")
            nc.vector.memset(ones, 1.0 / H)  # 2^-11, exact in bf16
            eps_t = consts.tile([128, 1], F32, tag="eps")
            nc.vector.memset(eps_t, EPS_LN)
            b1msb = consts.tile([1, E], F32, tag="b1m")
            nc.sync.dma_start(out=b1msb, in_=bcast(b1m.ap(), p=1))
            b2sb = consts.tile([E, D], BF16, tag="b2")
            nc.sync.dma_start(out=b2sb, in_=b2[:, :])
            cmb2sb = consts.tile([E, NS], BF16, tag="cmb2")
            nc.sync.dma_start(out=cmb2sb, in_=cmb2[:, :])

            y_tiles = []
            for i in range(NSLOT):
                y_tiles.append(
                    consts.tile([128, D], BF16, tag=f"y_{i}", name=f"y_{i}")
                )

            # ---------- experts (software-pipelined: mm2 of expert e-1
            # is emitted during expert e so the PE queue never stalls) ----

            def emit_mm2(unit):
                if unit is None:
                    return
                ee, hts, w2t = unit
                g0 = ee * C
                pos = g0
                while pos < g0 + C:
                    nxt = min(g0 + C, (pos // 128 + 1) * 128)
                    sz = nxt - pos
                    ls = pos - g0
                    y_ps = psY.tile([128, D], F32, tag="y")
                    for hk in range(HT):
                        nc.tensor.matmul(
                            y_ps[:sz, :],
                            hts[hk][:, ls:ls + sz],
                            w2t[:, hk, :],
                            start=(hk == 0),
                            stop=(hk == HT - 1),
                        )
                    po = pos % 128
                    nc.vector.tensor_copy(
                        y_tiles[pos // 128][po:po + sz, :], y_ps[:sz, :]
                    )
                    pos = nxt

            def emit_stage2(st):
                # stats + normalize for expert st (runs one block later)
                ee = st["e"]
                nc.tensor.matmul(st["sq_ps"], ones, st["sq_acc"], start=True, stop=True)
                mrow = rows.tile([1, C], F32, tag="mrow")
                nc.vector.tensor_scalar(
                    mrow, st["mean_ps"], b1msb[:, ee:ee + 1], None, ALU.add
                )
                rtmp = rows.tile([1, C], F32, tag="rtmp")
                nc.vector.tensor_mul(rtmp, mrow, mrow)
                nc.vector.tensor_sub(rtmp, st["sq_ps"], rtmp)  # var
                rstd0 = rows.tile([1, C], F32, tag="rstd0")
                nc.scalar.activation(rstd0, rtmp, AF.Sqrt, bias=eps_t[:1, :])
                rrstd = rows.tile([1, C], F32, tag="rrstd")
                nc.vector.reciprocal(rrstd, rstd0)
                r_row = rows.tile([1, C], BF16, tag="rrow")
                nc.vector.tensor_copy(r_row, rrstd)
                m2_row = rows.tile([1, C], BF16, tag="m2row")
                nc.vector.tensor_mul(m2_row, mrow, rrstd)
                r_b = bcastp.tile([128, C], BF16, tag="rb")
                nc.gpsimd.partition_broadcast(r_b, r_row)
                m2_b = bcastp.tile([128, C], BF16, tag="m2b")
                nc.gpsimd.partition_broadcast(m2_b, m2_row)
                for hi in range(HT):
                    h_sb = st["h_tiles"][hi]
                    t = work.tile([128, C], BF16, tag="t1")
                    nc.vector.tensor_mul(t, h_sb, r_b)
                    nc.vector.tensor_sub(h_sb, t, m2_b)
                    nc.scalar.activation(
                        h_sb,
                        h_sb,
                        AF.Gelu,
                        bias=st["lnbsb"][:, hi:hi + 1],
                        scale=st["lngsb"][:, hi:hi + 1],
                    )
                return (ee, st["h_tiles"], st["w2sb"])

            pending = None    # stage-1 done (mm1/hsq), awaiting stats+norm
            prev_unit = None  # stage-2 done, awaiting mm2
            for e in range(E):
                w1sb = []
                for k in range(KD):
                    t = wpool.tile([128, H], BF16, tag=f"w1_{k}", name=f"w1_{k}")
                    nc.sync.dma_start(
                        out=t, in_=w1[e, k * 128:(k + 1) * 128, :]
                    )
                    w1sb.append(t)
                xgsb = []
                for k in range(KD):
                    t = wpool.tile([128, C], BF16, tag=f"xg_{k}", name=f"xg_{k}")
                    nc.sync.dma_start(
                        out=t, in_=xgT[e, k * 128:(k + 1) * 128, :]
                    )
                    xgsb.append(t)
                b1sb = wpool.tile([128, HT], F32, tag="b1")
                nc.sync.dma_start(out=b1sb, in_=b1[e].rearrange("(i p) -> p i", p=128))
                lngsb = wpool.tile([128, HT], F32, tag="lng")
                nc.sync.dma_start(out=lngsb, in_=lng[e].rearrange("(i p) -> p i", p=128))
                lnbsb = wpool.tile([128, HT], F32, tag="lnb")
                nc.sync.dma_start(out=lnbsb, in_=lnb[e].rearrange("(i p) -> p i", p=128))
                w1msb = wpool.tile([128, KD], BF16, tag="w1m")
                nc.sync.dma_start(out=w1msb, in_=w1m[e].rearrange("(k p) -> p k", p=128))
                w2sb = w2pool.tile([128, HT, D], BF16, tag="w2")
                for half in range(2):
                    hh = HT // 2
                    nc.sync.dma_start(
                        out=w2sb[:, half * hh:(half + 1) * hh, :],
                        in_=w2[
                            e, half * (H // 2):(half + 1) * (H // 2), :
                        ].rearrange("(t p) d -> p t d", p=128),
                    )

                mean_ps = psStats.tile([1, C], F32, tag="sum")
                sq_ps = psStats.tile([1, C], F32, tag="sq")
                h_tiles = []
                sq_acc = hsqp.tile([128, C], BF16, tag="sqacc", bufs=2)

                for hi in range(HT):
                    h_ps = psA.tile([128, C], F32, tag="ps")
                    for k in range(KD):
                        nc.tensor.matmul(
                            h_ps,
                            w1sb[k][:, hi * 128:(hi + 1) * 128],
                            xgsb[k],
                            start=(k == 0),
                            stop=(k == KD - 1),
                        )
                    h_sb = hpool.tile([128, C], BF16, tag=f"h{hi}")
                    nc.scalar.activation(
                        h_sb, h_ps, AF.Identity, bias=b1sb[:, hi:hi + 1]
                    )
                    if hi == 0:
                        nc.vector.tensor_mul(sq_acc, h_sb, h_sb)
                    else:
                        hsq = hsqp.tile([128, C], BF16, tag="hsq")
                        nc.vector.tensor_mul(hsq, h_sb, h_sb)
                        nc.vector.tensor_add(sq_acc, sq_acc, hsq)
                    h_tiles.append(h_sb)

                unit = emit_stage2(pending) if pending is not None else None

                # mean = xg @ mean_H(W1[e]) + mean(b1[e])  (no h dependency)
                for k in range(KD):
                    nc.tensor.matmul(
                        mean_ps,
                        w1msb[:, k:k + 1],
                        xgsb[k],
                        start=(k == 0),
                        stop=(k == KD - 1),
                    )

                emit_mm2(prev_unit)
                prev_unit = unit
                pending = {
                    "e": e, "h_tiles": h_tiles, "sq_acc": sq_acc,
                    "mean_ps": mean_ps, "sq_ps": sq_ps,
                    "lngsb": lngsb, "lnbsb": lnbsb, "w2sb": w2sb,
                }

            last_unit = emit_stage2(pending)
            emit_mm2(prev_unit)
            emit_mm2(last_unit)

            emit_mm2(prev_unit)

            # ---------- combine (scatter-add as matmul) + final LN ----------
            for g in range(NTOK):
                msb = mpool.tile([128, NSLOT, 128], BF16, tag="m")
                nc.sync.dma_start(
                    out=msb,
                    in_=cm.ap().rearrange("(kt p) t -> p kt t", p=128)[
                        :, :, g * 128:(g + 1) * 128
                    ],
                )
                o_ps = psC.tile([128, D], F32, tag="oc")
                for kt in range(NSLOT):
                    nc.tensor.matmul(
                        o_ps,
                        msb[:, kt, :],
                        y_tiles[kt],
                        start=(kt == 0),
                        stop=False,
                    )
                # + b2 rows weighted by gating weights (K=E matmul)
                nc.tensor.matmul(
                    o_ps,
                    cmb2sb[:, g * 128:(g + 1) * 128],
                    b2sb,
                    start=False,
                    stop=True,
                )
                st6 = work.tile([128, 6], F32, tag="fst6")
                nc.vector.bn_stats(st6, o_ps)
                mv = work.tile([128, 2], F32, tag="fmv")
                nc.vector.bn_aggr(mv, st6)
                stdf = work.tile([128, 1], F32, tag="fstd")
                nc.scalar.activation(stdf, mv[:, 1:2], AF.Sqrt, bias=eps_t)
                rf = work.tile([128, 1], F32, tag="frf")
                nc.vector.reciprocal(rf, stdf)
                t = work.tile([128, D], F32, tag="fin")
                nc.vector.tensor_scalar(
                    t, o_ps, mv[:, 0:1], rf, ALU.subtract, ALU.mult
                )
                nc.vector.tensor_mul(t, t, og_b)
                nc.vector.tensor_add(t, t, ob_b)
                nc.sync.dma_start(out=out[g * 128:(g + 1) * 128, :], in_=t)

    nc.compile()
    return nc


def _get_nc(C):
    key = ("nc", C)
    if key not in _CACHE:
        _CACHE[key] = _build(C)
    return _CACHE[key]


def kernel(x, gate_W, gate_b, W1, b1, ln_g, ln_b, W2, b2, out_g, out_b):
    import os
    from concourse.bass_utils import run_bass_kernel_spmd

    x = np.asarray(x, dtype=np.float32)
    gate_W = np.asarray(gate_W, dtype=np.float32)
    gate_b = np.asarray(gate_b, dtype=np.float32)

    # ---------- host gating: softmax + top-2 + renormalize ----------
    logits = x @ gate_W + gate_b                      # [N, E] fp32
    lmax = logits.max(axis=1, keepdims=True)
    ex = np.exp((logits - lmax).astype(np.float32))
    probs = ex / ex.sum(axis=1, keepdims=True)        # [N, E] fp32
    order = np.argsort(-probs, axis=1, kind="stable")
    top_idx = order[:, :K]                            # [N, 2]
    top_p = np.take_along_axis(probs, top_idx, axis=1)
    top_w = (top_p / top_p.sum(axis=1, keepdims=True)).astype(np.float32)

    # aux loss (host; matches reference formulas)
    imp = probs.sum(axis=0).astype(np.float64)
    mask_count = np.zeros(E, dtype=np.float64)
    for kk in range(K):
        mask_count += np.bincount(top_idx[:, kk], minlength=E)
    load = mask_count / N

    def _loss(v):
        return (np.std(v, ddof=1) / (np.mean(v) + EPS_AUX)) ** 2

    aux = np.float32(_loss(imp) + _loss(load))

    # ---------- capacity ----------
    counts = np.zeros((NCORES, E), dtype=np.int64)
    for c in range(NCORES):
        ti = top_idx[c * NS:(c + 1) * NS]
        for e in range(E):
            counts[c, e] = int((ti == e).sum())
    C = max(320, int(np.ceil(counts.max() / 64.0) * 64))
    while (E * C) % 128 != 0:
        C += 64

    nc = _get_nc(C)

    # ---------- per-core routing buffers ----------
    bf16 = ml_dtypes.bfloat16
    W1f = np.asarray(W1, dtype=np.float32)
    b1f = np.ascontiguousarray(np.asarray(b1, dtype=np.float32))
    common = {
        "w1": np.ascontiguousarray(W1f).astype(bf16),
        "b1": b1f,
        "lng": np.ascontiguousarray(np.asarray(ln_g, dtype=np.float32)),
        "lnb": np.ascontiguousarray(np.asarray(ln_b, dtype=np.float32)),
        "w2": np.ascontiguousarray(np.asarray(W2, dtype=np.float32)).astype(bf16),
        "b2": np.ascontiguousarray(np.asarray(b2, dtype=np.float32)).astype(bf16),
        "w1m": np.ascontiguousarray(W1f.mean(axis=2)).astype(bf16),
        "b1m": np.ascontiguousarray(b1f.mean(axis=1)),
        "og": np.ascontiguousarray(np.asarray(out_g, dtype=np.float32)),
        "ob": np.ascontiguousarray(np.asarray(out_b, dtype=np.float32)),
    }

    in_maps = []
    for c in range(NCORES):
        sl = slice(c * NS, (c + 1) * NS)
        xs = x[sl]                                    # [NS, D]
        ti = top_idx[sl]                              # [NS, 2]
        tw = top_w[sl]
        xgT_c = np.zeros((E, D, C), dtype=bf16)
        cm_c = np.zeros((E * C, NS), dtype=bf16)
        cmb2_c = np.zeros((E, NS), dtype=bf16)
        for e in range(E):
            rows_e, which = np.nonzero(ti == e)
            ne = rows_e.shape[0]
            assert ne <= C, f"capacity overflow: {ne} > {C}"
            xgT_c[e, :, :ne] = xs[rows_e].T.astype(bf16)
            w = tw[rows_e, which].astype(bf16)
            cm_c[e * C + np.arange(ne), rows_e] = w
            cmb2_c[e, rows_e] = w
        in_maps.append(
            {**common, "xgT": xgT_c, "cm": cm_c, "cmb2": cmb2_c}
        )

    trace = bool(int(os.environ.get("BASS_KERNEL_TRACE", "0")))
    if trace:
        _install_ntff_hook()
    res = run_bass_kernel_spmd(
        nc, in_maps, core_ids=list(range(NCORES)), trace=trace
    )
    _CACHE["exec_time_ns"] = res.exec_time_ns

    out = np.concatenate([res.results[c]["out"] for c in range(NCORES)], axis=0)
    return out, aux


def _install_ntff_hook():
    import sys
    import types

    if "antenv.axon_hooks" in sys.modules:
        return
    mod = types.ModuleType("antenv.axon_hooks")
    hook = [None]
    mod.set_axon_ntff_profile_hook = lambda h: hook.__setitem__(0, h)
    mod.get_axon_ntff_profile_hook = lambda: hook[0]
    sys.modules["antenv.axon_hooks"] = mod
    try:
        import antenv

        antenv.axon_hooks = mod
        from trn_agent_boot.trn_boot import _ntff_profile_via_ctypes

        mod.set_axon_ntff_profile_hook(
            _ntff_profile_via_ctypes("/opt/axon/libaxon_pjrt.so")
        )
    except Exception:
        pass
